# revision 24
# baseline (speedup 1.0000x reference)
"""GraphUnpooling Trainium2 kernel.

Reference computation (per batch b):
    h  = x @ W.T + b                      # x: [NC, Fm, H], Linear over H
    hn = LayerNorm(h) * gamma + beta      # over H
    y  = silu(hn)                         # [NC, Fm, H]
    out[fi] = y[fine_to_coarse[fi]]       # broadcast to NF fine nodes

Shapes: B=4, NC=2000, Fm=8, H=128, NF=16000.  Output 256MB f32 -> memory bound.

Strategy (8 cores): core i handles batch b=i//2, fine half h=i%2 (8000 fine rows).
Each core streams the coarse table tile-by-tile (128 coarse nodes per tile,
each node = Fm*H = 1024 contiguous floats):
  - load x tile [128, 1024] (coarse on partitions)
  - per Fm-block k: PE-transpose [128,128] slice -> xT, matmul xT.T @ W.T -> PSUM
  - add bias, LayerNorm per (coarse, k) row via bn_stats/bn_aggr,
    fused normalize+SiLU on the scalar engine (per-partition scale/bias)
  - indirect-scatter the finished SBUF tile straight to the output rows that
    reference these coarse nodes (host precomputes, per coarse node, the list
    of fine slots in this core's half, padded to K with an out-of-bounds
    sentinel that the DMA skips).
This avoids any DRAM round-trip for the transformed table: per-core traffic is
~8.4MB read + ~32MB scattered 4KB-row writes, near the memory roofline.
"""

import numpy as np

B, NC, Fm, H = 4, 2000, 8, 128
NF = 16000
HALF = NF // 2  # fine rows per core
ROW = Fm * H  # 1024 floats per table row
P = 128
NCP = 2048  # coarse padded to multiple of P
NT = NCP // P  # 16 tiles
N_CORES = 8
LN_EPS = 1e-5

_PROGRAM_CACHE = {}


def _build_program(K, n_mat, with_gamma_beta):
    import concourse.bass as bass
    from concourse import bacc, mybir
    from concourse.tile import TileContext

    f32 = mybir.dt.float32
    i32 = mybir.dt.int32

    nc = bacc.Bacc("TRN2")

    x_d = nc.dram_tensor("x", [NCP, ROW], f32, kind="ExternalInput")
    idx_d = nc.dram_tensor("idx", [NCP, K], i32, kind="ExternalInput")
    wt_d = nc.dram_tensor("wt", [H, H], f32, kind="ExternalInput")  # W.T  [h, o]
    b1_d = nc.dram_tensor("b1", [1, H], f32, kind="ExternalInput")
    ones1_d = nc.dram_tensor("ones1", [1, H], f32, kind="ExternalInput")
    id_d = nc.dram_tensor("ident", [P, P], f32, kind="ExternalInput")
    if with_gamma_beta:
        gg_d = nc.dram_tensor("gg", [P, ROW], f32, kind="ExternalInput")
        be_d = nc.dram_tensor("be", [P, ROW], f32, kind="ExternalInput")
    # one extra garbage row at index HALF: padded scatter slots land there,
    # so no OOB bounds machinery is needed in the DMA
    out_d = nc.dram_tensor("out", [HALF + 1, ROW], f32, kind="ExternalOutput")

    with TileContext(nc) as tc:
        with (
            tc.tile_pool(name="singles", bufs=1) as singles,
            tc.tile_pool(name="inp", bufs=3) as inp,
            tc.tile_pool(name="idxp", bufs=3) as idxp,
            tc.tile_pool(name="xtp", bufs=3) as xtp,
            tc.tile_pool(name="hp", bufs=3) as hp,
            tc.tile_pool(name="outp", bufs=3) as outp,
            tc.tile_pool(name="statp", bufs=3) as statp,
            tc.tile_pool(name="ps_t", bufs=2, space="PSUM") as ps_t,
            tc.tile_pool(name="ps_h", bufs=1, space="PSUM") as ps_h,
            tc.tile_pool(name="ps_d", bufs=1, space="PSUM") as ps_d,
        ):
            wt_sb = singles.tile([H, H], f32)
            nc.sync.dma_start(out=wt_sb[:], in_=wt_d[:, :])
            ident = singles.tile([P, P], f32)
            nc.sync.dma_start(out=ident[:], in_=id_d[:, :])
            b1_sb = singles.tile([1, H], f32)
            nc.sync.dma_start(out=b1_sb[:], in_=b1_d[:, :])
            ones1 = singles.tile([1, H], f32)
            nc.sync.dma_start(out=ones1[:], in_=ones1_d[:, :])
            eps_sb = singles.tile([P, 1], f32)
            nc.vector.memset(eps_sb[:], LN_EPS)
            if with_gamma_beta:
                gg_sb = singles.tile([P, ROW], f32)
                nc.sync.dma_start(out=gg_sb[:], in_=gg_d[:, :])
                be_sb = singles.tile([P, ROW], f32)
                nc.sync.dma_start(out=be_sb[:], in_=be_d[:, :])

            # PE matmuls may carry at most ONE sync wait in codegen. Absorb
            # each constant's DMA-lane wait in a throwaway PE op so steady-state
            # matmuls only ever wait on the scalar engine (xT copy producer).
            psum_h = ps_h.tile([P, ROW], f32)
            dummy = ps_d.tile([P, P], f32)
            nc.tensor.transpose(out=dummy[:], in_=ident[:], identity=ident[:])
            nc.tensor.matmul(
                out=dummy[:], lhsT=wt_sb[:], rhs=wt_sb[:], start=True, stop=True
            )
            nc.tensor.matmul(
                out=dummy[:], lhsT=ones1[:], rhs=ones1[:], start=True, stop=True
            )
            nc.tensor.matmul(
                out=dummy[:], lhsT=b1_sb[:], rhs=b1_sb[:], start=True, stop=True
            )

            for t in range(NT):
                c0 = t * P
                in_tile = inp.tile([P, ROW], f32)
                nc.sync.dma_start(out=in_tile[:], in_=x_d[c0 : c0 + P, :])
                idx_tile = idxp.tile([P, K], i32)
                nc.sync.dma_start(out=idx_tile[:], in_=idx_d[c0 : c0 + P, :])

                for k in range(Fm):
                    blk = slice(k * H, (k + 1) * H)
                    psum_t = ps_t.tile([P, P], f32)
                    nc.tensor.transpose(
                        out=psum_t[:], in_=in_tile[:, blk], identity=ident[:]
                    )
                    xT = xtp.tile([P, P], f32)
                    nc.scalar.copy(out=xT[:], in_=psum_t[:])
                    # bias row via rank-1 matmul opens the accumulation group
                    nc.tensor.matmul(
                        out=psum_h[:, blk],
                        lhsT=ones1[:],
                        rhs=b1_sb[:],
                        start=True,
                        stop=False,
                    )
                    nc.tensor.matmul(
                        out=psum_h[:, blk],
                        lhsT=xT[:],
                        rhs=wt_sb[:],
                        start=False,
                        stop=True,
                    )

                # single PSUM reader (scalar engine) keeps matmul waits at one
                h_sb = hp.tile([P, ROW], f32)
                nc.scalar.copy(out=h_sb[:], in_=psum_h[:])

                # LayerNorm stats per (coarse partition, Fm block)
                stats = statp.tile([P, Fm, 6], f32)
                mv = statp.tile([P, Fm, 2], f32)
                for k in range(Fm):
                    blk = slice(k * H, (k + 1) * H)
                    nc.vector.bn_stats(out=stats[:, k, :], in_=h_sb[:, blk])
                    nc.vector.bn_aggr(out=mv[:, k, :], in_=stats[:, k, :])
                # rstd = 1/sqrt(var + eps); nmr = -mean * rstd
                rstd = statp.tile([P, Fm], f32)
                nc.scalar.activation(
                    out=rstd[:],
                    in_=mv[:, :, 1],
                    func=mybir.ActivationFunctionType.Sqrt,
                    bias=eps_sb[:],
                )
                nc.vector.reciprocal(out=rstd[:], in_=rstd[:])
                nmr = statp.tile([P, Fm], f32)
                nc.vector.tensor_mul(out=nmr[:], in0=mv[:, :, 0], in1=rstd[:])
                nc.vector.tensor_scalar_mul(out=nmr[:], in0=nmr[:], scalar1=-1.0)

                out_sb = outp.tile([P, ROW], f32)
                # absorber: first touch of the out_sb slot reads nmr, so it
                # carries both the scatter WAR wait and the DVE stats wait;
                # the silu ops below then stay within the 2-wait ACT limit.
                nc.scalar.activation(
                    out=out_sb[:, 0:1],
                    in_=nmr[:, 0:1],
                    func=mybir.ActivationFunctionType.Copy,
                )
                if not with_gamma_beta:
                    # silu((h - mu) * rstd), fused per Fm block on scalar engine
                    for k in range(Fm):
                        blk = slice(k * H, (k + 1) * H)
                        nc.scalar.activation(
                            out=out_sb[:, blk],
                            in_=h_sb[:, blk],
                            func=mybir.ActivationFunctionType.Silu,
                            scale=rstd[:, k : k + 1],
                            bias=nmr[:, k : k + 1],
                        )
                else:
                    tmp = outp.tile([P, ROW], f32, tag="gbtmp")
                    for k in range(Fm):
                        blk = slice(k * H, (k + 1) * H)
                        nc.scalar.activation(
                            out=tmp[:, blk],
                            in_=h_sb[:, blk],
                            func=mybir.ActivationFunctionType.Copy,
                            scale=rstd[:, k : k + 1],
                            bias=nmr[:, k : k + 1],
                        )
                    nc.vector.tensor_mul(out=tmp[:], in0=tmp[:], in1=gg_sb[:])
                    nc.vector.tensor_add(out=tmp[:], in0=tmp[:], in1=be_sb[:])
                    nc.scalar.activation(
                        out=out_sb[:],
                        in_=tmp[:],
                        func=mybir.ActivationFunctionType.Silu,
                    )

                # scatter finished rows to their fine slots. Rows are sorted
                # by fine-count descending (host-side), so the rows needing a
                # j-th copy form a partition prefix; any per-core shortfall
                # against the cross-core max lands in the garbage row HALF.
                for j in range(K):
                    n = n_mat[t][j]
                    if n == 0:
                        continue
                    nc.gpsimd.indirect_dma_start(
                        out=out_d[:, :],
                        out_offset=bass.IndirectOffsetOnAxis(
                            ap=idx_tile[:n, j : j + 1], axis=0
                        ),
                        in_=out_sb[:n],
                        in_offset=None,
                    )
    nc.finalize()
    return nc


def _prep_core_inputs(x_np, fine_to_coarse, Wt, b1, ones1, ident, gg, be, K, with_gb):
    """Build the 8 per-core input dicts."""
    in_maps = []
    for core in range(N_CORES):
        b = core // 2
        half = core % 2
        fc = fine_to_coarse[half * HALF : (half + 1) * HALF]
        counts = np.bincount(fc, minlength=NCP)
        starts = np.zeros(NCP + 1, dtype=np.int64)
        np.cumsum(counts, out=starts[1:])
        order = np.argsort(fc, kind="stable").astype(np.int32)
        sorted_c = fc[order]
        rank = np.arange(HALF, dtype=np.int64) - starts[sorted_c]
        idx_pad = np.full((NCP, K), HALF, dtype=np.int32)
        idx_pad[sorted_c, rank] = order

        x_pad = np.zeros((NCP, ROW), dtype=np.float32)
        x_pad[:NC] = x_np[b].reshape(NC, ROW)

        # sort coarse rows by fine-count descending so per-j scatters cover a
        # partition prefix only
        corder = np.argsort(-counts, kind="stable")
        x_pad = np.ascontiguousarray(x_pad[corder])
        idx_pad = np.ascontiguousarray(idx_pad[corder])

        m = {
            "x": x_pad,
            "idx": idx_pad,
            "wt": Wt,
            "b1": b1,
            "ones1": ones1,
            "ident": ident,
        }
        if with_gb:
            m["gg"] = gg
            m["be"] = be
        in_maps.append(m)
    return in_maps


def _run(x, W, b, gamma, beta, fine_to_coarse, num_fine_nodes, trace=False):
    from concourse.bass_utils import run_bass_kernel_spmd

    x_np = np.asarray(x, dtype=np.float32)
    W_np = np.asarray(W, dtype=np.float32)
    b_np = np.asarray(b, dtype=np.float32)
    g_np = np.asarray(gamma, dtype=np.float32)
    be_np = np.asarray(beta, dtype=np.float32)
    fc_np = np.asarray(fine_to_coarse, dtype=np.int32)

    assert x_np.shape == (B, NC, Fm, H)
    assert int(num_fine_nodes) == NF

    with_gb = not (
        np.all(g_np == 1.0) and np.all(be_np == 0.0)
    )

    # K = max fine-slot count of any (core, coarse node), shared by all cores.
    # n_mat[t][j] = max over cores of #rows in tile t (count-desc sorted)
    # needing a j-th copy — the SPMD program is shared, so take the max.
    K = 1
    for half in range(2):
        fc = fc_np[half * HALF : (half + 1) * HALF]
        K = max(K, int(np.bincount(fc, minlength=NC).max()))
    n_mat = np.zeros((NT, K), dtype=np.int64)
    for half in range(2):
        fc = fc_np[half * HALF : (half + 1) * HALF]
        counts = np.bincount(fc, minlength=NCP)
        cs = -np.sort(-counts)  # descending
        for t in range(NT):
            seg = cs[t * P : (t + 1) * P]
            for j in range(K):
                n_mat[t, j] = max(n_mat[t, j], int((seg > j).sum()))
    n_mat = tuple(tuple(int(v) for v in row) for row in n_mat)

    key = (K, n_mat, with_gb)
    if key not in _PROGRAM_CACHE:
        _PROGRAM_CACHE[key] = _build_program(K, n_mat, with_gb)
    nc = _PROGRAM_CACHE[key]

    Wt = np.ascontiguousarray(W_np.T)
    b1 = b_np.reshape(1, H).copy()
    ones1 = np.ones((1, H), dtype=np.float32)
    ident = np.eye(P, dtype=np.float32)
    gg = np.tile(g_np, (P, Fm)).astype(np.float32) if with_gb else None
    be = np.tile(be_np, (P, Fm)).astype(np.float32) if with_gb else None

    in_maps = _prep_core_inputs(x_np, fc_np, Wt, b1, ones1, ident, gg, be, K, with_gb)

    res = run_bass_kernel_spmd(nc, in_maps, list(range(N_CORES)), trace=trace)

    out = np.empty((B, NF, Fm, H), dtype=np.float32)
    for core in range(N_CORES):
        b_i = core // 2
        half = core % 2
        out[b_i, half * HALF : (half + 1) * HALF] = res.results[core]["out"][
            :HALF
        ].reshape(HALF, Fm, H)
    return out, res


def kernel(x, W, b, gamma, beta, fine_to_coarse, num_fine_nodes):
    out, _ = _run(x, W, b, gamma, beta, fine_to_coarse, num_fine_nodes)
    return out
